# revision 8
# baseline (speedup 1.0000x reference)
"""Gumbel-softmax hard VQ codebook kernel for 8 trn2 NeuronCores.

Computes, for inputs logits/gumbel [8, 4096, 2048] and codebook [2048, 256]:
    s   = logits + gumbel
    idx = argmax(s, axis=-1)
    y   = one_hot(idx, 2048) (f32)    # == fwd value of straight-through output
    z   = codebook[idx]               # == einsum(one_hot, codebook)

Sharding: data-parallel over the batch dim (1 batch row of 4096 tokens per core).
"""

import numpy as np

from concourse import bass, bacc, mybir
from concourse.tile import TileContext
from concourse.bass_utils import run_bass_kernel_spmd

P = 128
N_CODES = 2048
CODE_DIM = 256
TOKENS_PER_CORE = 4096
TPP = 1  # tokens per partition per tile
TILE_TOKENS = P * TPP  # 256
N_TILES = TOKENS_PER_CORE // TILE_TOKENS
N_CORES = 8

F32 = mybir.dt.float32
I32 = mybir.dt.int32
U32 = mybir.dt.uint32


def build_program() -> bass.Bass:
    nc = bacc.Bacc()
    logits = nc.declare_dram_parameter(
        "logits", [TOKENS_PER_CORE, N_CODES], F32, isOutput=False
    )
    gumbel = nc.declare_dram_parameter(
        "gumbel", [TOKENS_PER_CORE, N_CODES], F32, isOutput=False
    )
    codebook = nc.declare_dram_parameter(
        "codebook", [N_CODES, CODE_DIM], F32, isOutput=False
    )
    y_out = nc.declare_dram_parameter(
        "y", [TOKENS_PER_CORE, N_CODES], F32, isOutput=True
    )
    z_out = nc.declare_dram_parameter(
        "z", [TOKENS_PER_CORE, CODE_DIM], F32, isOutput=True
    )

    with TileContext(nc) as tc:
        with (
            tc.tile_pool(name="const", bufs=1) as cpool,
            tc.tile_pool(name="big", bufs=3) as pool,
            tc.tile_pool(name="small", bufs=2 * TPP + 2) as spool,
        ):
            iota_i = cpool.tile([P, N_CODES], I32)
            nc.gpsimd.iota(
                iota_i[:], pattern=[[1, N_CODES]], base=0, channel_multiplier=0
            )
            iota_f = cpool.tile([P, N_CODES], F32)
            nc.vector.tensor_copy(iota_f[:], iota_i[:])

            for t in range(N_TILES):
                r0 = t * TILE_TOKENS
                rows = slice(r0, r0 + TILE_TOKENS)
                s = pool.tile([P, TPP * N_CODES], F32, tag="s")
                nc.gpsimd.dma_start(out=s[:], in_=logits[rows, :])
                # s += gumbel, accumulated inline by the DMA (SWDGE CCE add)
                nc.gpsimd.dma_start(
                    out=s[:], in_=gumbel[rows, :], accum_op=mybir.AluOpType.add
                )
                y_tile = pool.tile([P, TPP * N_CODES], F32, tag="y")
                z_tile = pool.tile([P, TPP * CODE_DIM], F32, tag="z")
                for a in range(TPP):
                    sl = s[:, a * N_CODES : (a + 1) * N_CODES]
                    max8 = spool.tile([P, 8], F32, tag=f"max8_{a}")
                    idx8 = spool.tile([P, 8], U32, tag=f"idx8_{a}")
                    nc.vector.max(max8[:], sl)
                    nc.vector.max_index(idx8[:], max8[:], sl)
                    idxf = spool.tile([P, 1], F32, tag=f"idxf_{a}")
                    nc.vector.tensor_copy(idxf[:], idx8[:, 0:1])
                    nc.vector.tensor_scalar(
                        out=y_tile[:, a * N_CODES : (a + 1) * N_CODES],
                        in0=iota_f[:],
                        scalar1=idxf[:],
                        scalar2=None,
                        op0=mybir.AluOpType.is_equal,
                    )
                    nc.gpsimd.indirect_dma_start(
                        out=z_tile[:, a * CODE_DIM : (a + 1) * CODE_DIM],
                        out_offset=None,
                        in_=codebook[:],
                        in_offset=bass.IndirectOffsetOnAxis(ap=idx8[:, 0:1], axis=0),
                    )
                nc.sync.dma_start(out=y_out[rows, :], in_=y_tile[:])
                nc.sync.dma_start(out=z_out[rows, :], in_=z_tile[:])
    nc.finalize()
    return nc


_PROGRAM_CACHE = {}


def run_on_cores(logits, gumbel, codebook, trace=False, **trace_kwargs):
    if "nc" not in _PROGRAM_CACHE:
        _PROGRAM_CACHE["nc"] = build_program()
    nc = _PROGRAM_CACHE["nc"]
    logits = np.ascontiguousarray(np.asarray(logits, dtype=np.float32)).reshape(
        N_CORES, TOKENS_PER_CORE, N_CODES
    )
    gumbel = np.ascontiguousarray(np.asarray(gumbel, dtype=np.float32)).reshape(
        N_CORES, TOKENS_PER_CORE, N_CODES
    )
    codebook = np.ascontiguousarray(np.asarray(codebook, dtype=np.float32))
    in_maps = [
        {"logits": logits[i], "gumbel": gumbel[i], "codebook": codebook}
        for i in range(N_CORES)
    ]
    res = run_bass_kernel_spmd(
        nc, in_maps, list(range(N_CORES)), trace=trace, **trace_kwargs
    )
    z = np.stack([r["z"] for r in res.results]).astype(np.float32)
    y = np.stack([r["y"] for r in res.results]).astype(np.float32)
    return (z, y), res


def kernel(logits, gumbel, codebook):
    (z, y), _ = run_on_cores(logits, gumbel, codebook, trace=False)
    return (z, y)


# revision 11
# speedup vs baseline: 1.1224x; 1.1224x over previous
"""Gumbel-softmax hard VQ codebook kernel for 8 trn2 NeuronCores.

Computes, for inputs logits/gumbel [8, 4096, 2048] and codebook [2048, 256]:
    s   = logits + gumbel
    idx = argmax(s, axis=-1)
    y   = one_hot(idx, 2048) (f32)    # == fwd value of straight-through output
    z   = codebook[idx]               # == einsum(one_hot, codebook)

Sharding: data-parallel over the batch dim (1 batch row of 4096 tokens per core).

Implementation notes:
  - s is formed by a SWDGE DMA-accumulate (CCE add) during the gumbel load.
  - argmax via DVE max/max_index (top-8), first-occurrence tie-break matches
    jnp.argmax.
  - y is written sparsely: run_bass_kernel_spmd pre-zeros (donates zeroed)
    output buffers, so only the 1.0 at (token, idx) is scattered via
    indirect DMA (flat element offsets) - saves 32MB/core of DMA writes.
  - z rows are gathered from the codebook in DRAM via indirect DMA.
"""

import numpy as np

from concourse import bass, bacc, mybir
from concourse.tile import TileContext
from concourse.bass_utils import run_bass_kernel_spmd

P = 128
N_CODES = 2048
CODE_DIM = 256
TOKENS_PER_CORE = 4096
TILE_TOKENS = P
N_TILES = TOKENS_PER_CORE // TILE_TOKENS
N_CORES = 8

F32 = mybir.dt.float32
I32 = mybir.dt.int32
U32 = mybir.dt.uint32


def build_program() -> bass.Bass:
    nc = bacc.Bacc()
    logits = nc.declare_dram_parameter(
        "logits", [TOKENS_PER_CORE, N_CODES], F32, isOutput=False
    )
    gumbel = nc.declare_dram_parameter(
        "gumbel", [TOKENS_PER_CORE, N_CODES], F32, isOutput=False
    )
    codebook = nc.declare_dram_parameter(
        "codebook", [N_CODES, CODE_DIM], F32, isOutput=False
    )
    # y is flat so the one-hot can be scattered at element granularity.
    y_out = nc.declare_dram_parameter(
        "y", [TOKENS_PER_CORE * N_CODES, 1], F32, isOutput=True
    )
    z_out = nc.declare_dram_parameter(
        "z", [TOKENS_PER_CORE, CODE_DIM], F32, isOutput=True
    )

    with TileContext(nc) as tc:
        with (
            tc.tile_pool(name="const", bufs=1) as cpool,
            tc.tile_pool(name="big", bufs=4) as pool,
            tc.tile_pool(name="small", bufs=4) as spool,
        ):
            ones = cpool.tile([P, 1], F32)
            nc.vector.memset(ones[:], 1.0)

            for t in range(N_TILES):
                r0 = t * TILE_TOKENS
                rows = slice(r0, r0 + TILE_TOKENS)
                s = pool.tile([P, N_CODES], F32, tag="s")
                nc.gpsimd.dma_start(out=s[:], in_=logits[rows, :])
                # s += gumbel, accumulated inline by the DMA (SWDGE CCE add)
                nc.gpsimd.dma_start(
                    out=s[:], in_=gumbel[rows, :], accum_op=mybir.AluOpType.add
                )
                max8 = spool.tile([P, 8], F32, tag="max8")
                idx8 = spool.tile([P, 8], U32, tag="idx8")
                nc.vector.max(max8[:], s[:])
                nc.vector.max_index(idx8[:], max8[:], s[:])
                # flat offset of the argmax element: (r0 + p) * N_CODES + idx
                rowbase = spool.tile([P, 1], U32, tag="rowbase")
                nc.gpsimd.iota(
                    rowbase[:],
                    pattern=[[0, 1]],
                    base=r0 * N_CODES,
                    channel_multiplier=N_CODES,
                )
                off = spool.tile([P, 1], U32, tag="off")
                nc.vector.tensor_tensor(
                    out=off[:],
                    in0=idx8[:, 0:1],
                    in1=rowbase[:],
                    op=mybir.AluOpType.add,
                )
                nc.gpsimd.indirect_dma_start(
                    out=y_out[:],
                    out_offset=bass.IndirectOffsetOnAxis(ap=off[:, 0:1], axis=0),
                    in_=ones[:],
                    in_offset=None,
                )
                z_tile = pool.tile([P, CODE_DIM], F32, tag="z")
                nc.gpsimd.indirect_dma_start(
                    out=z_tile[:],
                    out_offset=None,
                    in_=codebook[:],
                    in_offset=bass.IndirectOffsetOnAxis(ap=idx8[:, 0:1], axis=0),
                )
                nc.sync.dma_start(out=z_out[rows, :], in_=z_tile[:])
    nc.finalize()
    return nc


_PROGRAM_CACHE = {}


def run_on_cores(logits, gumbel, codebook, trace=False, **trace_kwargs):
    if "nc" not in _PROGRAM_CACHE:
        _PROGRAM_CACHE["nc"] = build_program()
    nc = _PROGRAM_CACHE["nc"]
    logits = np.ascontiguousarray(np.asarray(logits, dtype=np.float32)).reshape(
        N_CORES, TOKENS_PER_CORE, N_CODES
    )
    gumbel = np.ascontiguousarray(np.asarray(gumbel, dtype=np.float32)).reshape(
        N_CORES, TOKENS_PER_CORE, N_CODES
    )
    codebook = np.ascontiguousarray(np.asarray(codebook, dtype=np.float32))
    in_maps = [
        {"logits": logits[i], "gumbel": gumbel[i], "codebook": codebook}
        for i in range(N_CORES)
    ]
    res = run_bass_kernel_spmd(
        nc, in_maps, list(range(N_CORES)), trace=trace, **trace_kwargs
    )
    z = np.stack([r["z"] for r in res.results]).astype(np.float32)
    y = np.stack(
        [r["y"].reshape(TOKENS_PER_CORE, N_CODES) for r in res.results]
    ).astype(np.float32)
    return (z, y), res


def kernel(logits, gumbel, codebook):
    (z, y), _ = run_on_cores(logits, gumbel, codebook, trace=False)
    return (z, y)


# revision 12
# speedup vs baseline: 1.1250x; 1.0023x over previous
"""Gumbel-softmax hard VQ codebook kernel for 8 trn2 NeuronCores.

Computes, for inputs logits/gumbel [8, 4096, 2048] and codebook [2048, 256]:
    s   = logits + gumbel
    idx = argmax(s, axis=-1)
    y   = one_hot(idx, 2048) (f32)    # == fwd value of straight-through output
    z   = codebook[idx]               # == einsum(one_hot, codebook)

Sharding: data-parallel over the batch dim (1 batch row of 4096 tokens per core).

Implementation notes:
  - Bulk loads go through HWDGE (nc.sync) with 2 tokens per partition so each
    DMA descriptor moves 16KB contiguous; the add runs on gpsimd (DVE does
    max/max_index).
  - argmax via DVE max/max_index (top-8), first-occurrence tie-break matches
    jnp.argmax.
  - y is written sparsely: run_bass_kernel_spmd pre-zeros (donates zeroed)
    output buffers, so only the 1.0 at (token, idx) is scattered via
    indirect DMA (flat element offsets) - saves 32MB/core of DMA writes.
  - z rows are gathered from the codebook in DRAM via indirect DMA (SWDGE).
"""

import numpy as np

from concourse import bass, bacc, mybir
from concourse.tile import TileContext
from concourse.bass_utils import run_bass_kernel_spmd

P = 128
N_CODES = 2048
CODE_DIM = 256
TOKENS_PER_CORE = 4096
TPP = 2  # tokens per partition per tile
TILE_TOKENS = P * TPP
N_TILES = TOKENS_PER_CORE // TILE_TOKENS
N_CORES = 8

F32 = mybir.dt.float32
I32 = mybir.dt.int32
U32 = mybir.dt.uint32


def build_program() -> bass.Bass:
    nc = bacc.Bacc()
    logits = nc.declare_dram_parameter(
        "logits", [TOKENS_PER_CORE, N_CODES], F32, isOutput=False
    )
    gumbel = nc.declare_dram_parameter(
        "gumbel", [TOKENS_PER_CORE, N_CODES], F32, isOutput=False
    )
    codebook = nc.declare_dram_parameter(
        "codebook", [N_CODES, CODE_DIM], F32, isOutput=False
    )
    # y is flat so the one-hot can be scattered at element granularity.
    y_out = nc.declare_dram_parameter(
        "y", [TOKENS_PER_CORE * N_CODES, 1], F32, isOutput=True
    )
    z_out = nc.declare_dram_parameter(
        "z", [TOKENS_PER_CORE, CODE_DIM], F32, isOutput=True
    )

    with TileContext(nc) as tc:
        with (
            tc.tile_pool(name="const", bufs=1) as cpool,
            tc.tile_pool(name="big", bufs=3) as pool,
            tc.tile_pool(name="small", bufs=4) as spool,
        ):
            ones = cpool.tile([P, 1], F32)
            nc.vector.memset(ones[:], 1.0)

            for t in range(N_TILES):
                r0 = t * TILE_TOKENS
                rows = slice(r0, r0 + TILE_TOKENS)
                # partition p holds tokens r0+2p (cols 0:2048) and r0+2p+1
                sl = pool.tile([P, TPP * N_CODES], F32, tag="sl")
                sg = pool.tile([P, TPP * N_CODES], F32, tag="sg")
                nc.sync.dma_start(
                    out=sl[:],
                    in_=logits[rows, :].rearrange("(p a) n -> p (a n)", p=P),
                )
                nc.sync.dma_start(
                    out=sg[:],
                    in_=gumbel[rows, :].rearrange("(p a) n -> p (a n)", p=P),
                )
                s = pool.tile([P, TPP * N_CODES], F32, tag="s")
                nc.gpsimd.tensor_tensor(
                    out=s[:], in0=sl[:], in1=sg[:], op=mybir.AluOpType.add
                )
                z_tile = pool.tile([P, TPP * CODE_DIM], F32, tag="z")
                for a in range(TPP):
                    blk = s[:, a * N_CODES : (a + 1) * N_CODES]
                    max8 = spool.tile([P, 8], F32, tag=f"max8_{a}")
                    idx8 = spool.tile([P, 8], U32, tag=f"idx8_{a}")
                    nc.vector.max(max8[:], blk)
                    nc.vector.max_index(idx8[:], max8[:], blk)
                    # flat offset of the argmax element:
                    #   (r0 + 2p + a) * N_CODES + idx
                    rowbase = spool.tile([P, 1], U32, tag=f"rowbase_{a}")
                    nc.gpsimd.iota(
                        rowbase[:],
                        pattern=[[0, 1]],
                        base=(r0 + a) * N_CODES,
                        channel_multiplier=TPP * N_CODES,
                    )
                    off = spool.tile([P, 1], U32, tag=f"off_{a}")
                    nc.vector.tensor_tensor(
                        out=off[:],
                        in0=idx8[:, 0:1],
                        in1=rowbase[:],
                        op=mybir.AluOpType.add,
                    )
                    nc.gpsimd.indirect_dma_start(
                        out=y_out[:],
                        out_offset=bass.IndirectOffsetOnAxis(ap=off[:, 0:1], axis=0),
                        in_=ones[:],
                        in_offset=None,
                    )
                    nc.gpsimd.indirect_dma_start(
                        out=z_tile[:, a * CODE_DIM : (a + 1) * CODE_DIM],
                        out_offset=None,
                        in_=codebook[:],
                        in_offset=bass.IndirectOffsetOnAxis(ap=idx8[:, 0:1], axis=0),
                    )
                nc.sync.dma_start(
                    out=z_out[rows, :].rearrange("(p a) n -> p (a n)", p=P),
                    in_=z_tile[:],
                )
    nc.finalize()
    return nc


_PROGRAM_CACHE = {}


def run_on_cores(logits, gumbel, codebook, trace=False, **trace_kwargs):
    if "nc" not in _PROGRAM_CACHE:
        _PROGRAM_CACHE["nc"] = build_program()
    nc = _PROGRAM_CACHE["nc"]
    logits = np.ascontiguousarray(np.asarray(logits, dtype=np.float32)).reshape(
        N_CORES, TOKENS_PER_CORE, N_CODES
    )
    gumbel = np.ascontiguousarray(np.asarray(gumbel, dtype=np.float32)).reshape(
        N_CORES, TOKENS_PER_CORE, N_CODES
    )
    codebook = np.ascontiguousarray(np.asarray(codebook, dtype=np.float32))
    in_maps = [
        {"logits": logits[i], "gumbel": gumbel[i], "codebook": codebook}
        for i in range(N_CORES)
    ]
    res = run_bass_kernel_spmd(
        nc, in_maps, list(range(N_CORES)), trace=trace, **trace_kwargs
    )
    z = np.stack([r["z"] for r in res.results]).astype(np.float32)
    y = np.stack(
        [r["y"].reshape(TOKENS_PER_CORE, N_CODES) for r in res.results]
    ).astype(np.float32)
    return (z, y), res


def kernel(logits, gumbel, codebook):
    (z, y), _ = run_on_cores(logits, gumbel, codebook, trace=False)
    return (z, y)


# revision 14
# speedup vs baseline: 1.2404x; 1.1026x over previous
"""Gumbel-softmax hard VQ codebook kernel for 8 trn2 NeuronCores.

Computes, for inputs logits/gumbel [8, 4096, 2048] and codebook [2048, 256]:
    s   = logits + gumbel
    idx = argmax(s, axis=-1)
    y   = one_hot(idx, 2048) (f32)    # == fwd value of straight-through output
    z   = codebook[idx]               # == einsum(one_hot, codebook)

Sharding: data-parallel over the batch dim (1 batch row of 4096 tokens per core).

Implementation notes:
  - Bulk loads go through HWDGE (nc.sync) with 2 tokens per partition so each
    DMA descriptor moves 16KB contiguous; the add runs on gpsimd (DVE does
    max/max_index).
  - argmax via DVE max/max_index (top-8), first-occurrence tie-break matches
    jnp.argmax.
  - y is written sparsely: run_bass_kernel_spmd pre-zeros (donates zeroed)
    output buffers, so only the 1.0 at (token, idx) is scattered via
    indirect DMA (flat element offsets) - saves 32MB/core of DMA writes.
  - z rows are gathered from the codebook in DRAM via indirect DMA (SWDGE).
"""

import numpy as np

from concourse import bass, bacc, mybir
from concourse.tile import TileContext
from concourse.bass_utils import run_bass_kernel_spmd

P = 128
N_CODES = 2048
CODE_DIM = 256
TOKENS_PER_CORE = 4096
TPP = 2  # tokens per partition per tile
TILE_TOKENS = P * TPP
N_TILES = TOKENS_PER_CORE // TILE_TOKENS
N_CORES = 8

F32 = mybir.dt.float32
I32 = mybir.dt.int32
U32 = mybir.dt.uint32


def build_program() -> bass.Bass:
    nc = bacc.Bacc()
    logits = nc.declare_dram_parameter(
        "logits", [TOKENS_PER_CORE, N_CODES], F32, isOutput=False
    )
    gumbel = nc.declare_dram_parameter(
        "gumbel", [TOKENS_PER_CORE, N_CODES], F32, isOutput=False
    )
    codebook = nc.declare_dram_parameter(
        "codebook", [N_CODES, CODE_DIM], F32, isOutput=False
    )
    # y is flat so the one-hot can be scattered at element granularity.
    y_out = nc.declare_dram_parameter(
        "y", [TOKENS_PER_CORE * N_CODES, 1], F32, isOutput=True
    )
    z_out = nc.declare_dram_parameter(
        "z", [TOKENS_PER_CORE, CODE_DIM], F32, isOutput=True
    )

    with TileContext(nc) as tc:
        with (
            tc.tile_pool(name="const", bufs=1) as cpool,
            tc.tile_pool(name="big", bufs=4) as pool,
            tc.tile_pool(name="small", bufs=6) as spool,
        ):
            ones = cpool.tile([P, 1], F32)
            nc.vector.memset(ones[:], 1.0)

            for t in range(N_TILES):
                r0 = t * TILE_TOKENS
                rows = slice(r0, r0 + TILE_TOKENS)
                # partition p holds tokens r0+2p (cols 0:2048) and r0+2p+1
                sl = pool.tile([P, TPP * N_CODES], F32, tag="sl")
                sg = pool.tile([P, TPP * N_CODES], F32, tag="sg")
                nc.sync.dma_start(
                    out=sl[:],
                    in_=logits[rows, :].rearrange("(p a) n -> p (a n)", p=P),
                )
                nc.sync.dma_start(
                    out=sg[:],
                    in_=gumbel[rows, :].rearrange("(p a) n -> p (a n)", p=P),
                )
                # add in place (s aliases sl); split across DVE and gpsimd to
                # balance engine load (DVE also runs max/max_index)
                s = sl
                H = TPP * N_CODES // 2
                nc.vector.tensor_tensor(
                    out=s[:, 0:H], in0=sl[:, 0:H], in1=sg[:, 0:H],
                    op=mybir.AluOpType.add,
                )
                nc.gpsimd.tensor_tensor(
                    out=s[:, H:], in0=sl[:, H:], in1=sg[:, H:],
                    op=mybir.AluOpType.add,
                )
                z_tile = pool.tile([P, TPP * CODE_DIM], F32, tag="z")
                for a in range(TPP):
                    blk = s[:, a * N_CODES : (a + 1) * N_CODES]
                    max8 = spool.tile([P, 8], F32, tag=f"max8_{a}")
                    idx8 = spool.tile([P, 8], U32, tag=f"idx8_{a}")
                    nc.vector.max(max8[:], blk)
                    nc.vector.max_index(idx8[:], max8[:], blk)
                    # flat offset of the argmax element:
                    #   (r0 + 2p + a) * N_CODES + idx
                    rowbase = spool.tile([P, 1], U32, tag=f"rowbase_{a}")
                    nc.gpsimd.iota(
                        rowbase[:],
                        pattern=[[0, 1]],
                        base=(r0 + a) * N_CODES,
                        channel_multiplier=TPP * N_CODES,
                    )
                    off = spool.tile([P, 1], U32, tag=f"off_{a}")
                    nc.vector.tensor_tensor(
                        out=off[:],
                        in0=idx8[:, 0:1],
                        in1=rowbase[:],
                        op=mybir.AluOpType.add,
                    )
                    nc.gpsimd.indirect_dma_start(
                        out=y_out[:],
                        out_offset=bass.IndirectOffsetOnAxis(ap=off[:, 0:1], axis=0),
                        in_=ones[:],
                        in_offset=None,
                    )
                    nc.gpsimd.indirect_dma_start(
                        out=z_tile[:, a * CODE_DIM : (a + 1) * CODE_DIM],
                        out_offset=None,
                        in_=codebook[:],
                        in_offset=bass.IndirectOffsetOnAxis(ap=idx8[:, 0:1], axis=0),
                    )
                nc.sync.dma_start(
                    out=z_out[rows, :].rearrange("(p a) n -> p (a n)", p=P),
                    in_=z_tile[:],
                )
    nc.finalize()
    return nc


_PROGRAM_CACHE = {}


def run_on_cores(logits, gumbel, codebook, trace=False, **trace_kwargs):
    if "nc" not in _PROGRAM_CACHE:
        _PROGRAM_CACHE["nc"] = build_program()
    nc = _PROGRAM_CACHE["nc"]
    logits = np.ascontiguousarray(np.asarray(logits, dtype=np.float32)).reshape(
        N_CORES, TOKENS_PER_CORE, N_CODES
    )
    gumbel = np.ascontiguousarray(np.asarray(gumbel, dtype=np.float32)).reshape(
        N_CORES, TOKENS_PER_CORE, N_CODES
    )
    codebook = np.ascontiguousarray(np.asarray(codebook, dtype=np.float32))
    in_maps = [
        {"logits": logits[i], "gumbel": gumbel[i], "codebook": codebook}
        for i in range(N_CORES)
    ]
    res = run_bass_kernel_spmd(
        nc, in_maps, list(range(N_CORES)), trace=trace, **trace_kwargs
    )
    z = np.stack([r["z"] for r in res.results]).astype(np.float32)
    y = np.stack(
        [r["y"].reshape(TOKENS_PER_CORE, N_CODES) for r in res.results]
    ).astype(np.float32)
    return (z, y), res


def kernel(logits, gumbel, codebook):
    (z, y), _ = run_on_cores(logits, gumbel, codebook, trace=False)
    return (z, y)


# revision 15
# speedup vs baseline: 1.2451x; 1.0038x over previous
"""Gumbel-softmax hard VQ codebook kernel for 8 trn2 NeuronCores.

Computes, for inputs logits/gumbel [8, 4096, 2048] and codebook [2048, 256]:
    s   = logits + gumbel
    idx = argmax(s, axis=-1)
    y   = one_hot(idx, 2048) (f32)    # == fwd value of straight-through output
    z   = codebook[idx]               # == einsum(one_hot, codebook)

Sharding: data-parallel over the batch dim (1 batch row of 4096 tokens per core).

Implementation notes:
  - Bulk loads go through HWDGE (nc.sync) with 2 tokens per partition so each
    DMA descriptor moves 16KB contiguous; the add runs on gpsimd (DVE does
    max/max_index).
  - argmax via DVE max/max_index (top-8), first-occurrence tie-break matches
    jnp.argmax.
  - y is written sparsely: run_bass_kernel_spmd pre-zeros (donates zeroed)
    output buffers, so only the 1.0 at (token, idx) is scattered via
    indirect DMA (flat element offsets) - saves 32MB/core of DMA writes.
  - z rows are gathered from the codebook in DRAM via indirect DMA (SWDGE).
"""

import numpy as np

from concourse import bass, bacc, mybir
from concourse.tile import TileContext
from concourse.bass_utils import run_bass_kernel_spmd

P = 128
N_CODES = 2048
CODE_DIM = 256
TOKENS_PER_CORE = 4096
TPP = 2  # tokens per partition per tile
TILE_TOKENS = P * TPP
N_TILES = TOKENS_PER_CORE // TILE_TOKENS
N_CORES = 8

F32 = mybir.dt.float32
I32 = mybir.dt.int32
U32 = mybir.dt.uint32


def build_program() -> bass.Bass:
    nc = bacc.Bacc()
    logits = nc.declare_dram_parameter(
        "logits", [TOKENS_PER_CORE, N_CODES], F32, isOutput=False
    )
    gumbel = nc.declare_dram_parameter(
        "gumbel", [TOKENS_PER_CORE, N_CODES], F32, isOutput=False
    )
    codebook = nc.declare_dram_parameter(
        "codebook", [N_CODES, CODE_DIM], F32, isOutput=False
    )
    # y is flat so the one-hot can be scattered at element granularity.
    y_out = nc.declare_dram_parameter(
        "y", [TOKENS_PER_CORE * N_CODES, 1], F32, isOutput=True
    )
    z_out = nc.declare_dram_parameter(
        "z", [TOKENS_PER_CORE, CODE_DIM], F32, isOutput=True
    )

    with TileContext(nc) as tc:
        with (
            tc.tile_pool(name="const", bufs=1) as cpool,
            tc.tile_pool(name="big", bufs=4) as pool,
            tc.tile_pool(name="small", bufs=6) as spool,
        ):
            ones = cpool.tile([P, 1], F32)
            nc.vector.memset(ones[:], 1.0)

            def emit_tail(item):
                """Indirect scatter/gather + z write for an already-argmaxed
                tile. Emitted one iteration late so the in-order gpsimd queue
                never stalls on the DVE argmax chain of the current tile."""
                t, offs, idx8s = item
                r0 = t * TILE_TOKENS
                rows = slice(r0, r0 + TILE_TOKENS)
                z_tile = pool.tile([P, TPP * CODE_DIM], F32, tag="z")
                for a in range(TPP):
                    nc.gpsimd.indirect_dma_start(
                        out=y_out[:],
                        out_offset=bass.IndirectOffsetOnAxis(
                            ap=offs[a][:, 0:1], axis=0
                        ),
                        in_=ones[:],
                        in_offset=None,
                    )
                    nc.gpsimd.indirect_dma_start(
                        out=z_tile[:, a * CODE_DIM : (a + 1) * CODE_DIM],
                        out_offset=None,
                        in_=codebook[:],
                        in_offset=bass.IndirectOffsetOnAxis(
                            ap=idx8s[a][:, 0:1], axis=0
                        ),
                    )
                nc.sync.dma_start(
                    out=z_out[rows, :].rearrange("(p a) n -> p (a n)", p=P),
                    in_=z_tile[:],
                )

            pending = None
            for t in range(N_TILES):
                r0 = t * TILE_TOKENS
                rows = slice(r0, r0 + TILE_TOKENS)
                # partition p holds tokens r0+2p (cols 0:2048) and r0+2p+1
                sl = pool.tile([P, TPP * N_CODES], F32, tag="sl")
                sg = pool.tile([P, TPP * N_CODES], F32, tag="sg")
                nc.sync.dma_start(
                    out=sl[:],
                    in_=logits[rows, :].rearrange("(p a) n -> p (a n)", p=P),
                )
                nc.sync.dma_start(
                    out=sg[:],
                    in_=gumbel[rows, :].rearrange("(p a) n -> p (a n)", p=P),
                )
                # add in place (s aliases sl); split across DVE and gpsimd to
                # balance engine load (DVE also runs max/max_index)
                s = sl
                H = TPP * N_CODES // 2
                nc.gpsimd.tensor_tensor(
                    out=s[:, H:], in0=sl[:, H:], in1=sg[:, H:],
                    op=mybir.AluOpType.add,
                )
                nc.vector.tensor_tensor(
                    out=s[:, 0:H], in0=sl[:, 0:H], in1=sg[:, 0:H],
                    op=mybir.AluOpType.add,
                )
                offs, idx8s = [], []
                for a in range(TPP):
                    blk = s[:, a * N_CODES : (a + 1) * N_CODES]
                    max8 = spool.tile([P, 8], F32, tag=f"max8_{a}")
                    idx8 = spool.tile([P, 8], U32, tag=f"idx8_{a}")
                    nc.vector.max(max8[:], blk)
                    nc.vector.max_index(idx8[:], max8[:], blk)
                    # flat offset of the argmax element:
                    #   (r0 + 2p + a) * N_CODES + idx
                    rowbase = spool.tile([P, 1], U32, tag=f"rowbase_{a}")
                    nc.gpsimd.iota(
                        rowbase[:],
                        pattern=[[0, 1]],
                        base=(r0 + a) * N_CODES,
                        channel_multiplier=TPP * N_CODES,
                    )
                    off = spool.tile([P, 1], U32, tag=f"off_{a}")
                    nc.vector.tensor_tensor(
                        out=off[:],
                        in0=idx8[:, 0:1],
                        in1=rowbase[:],
                        op=mybir.AluOpType.add,
                    )
                    offs.append(off)
                    idx8s.append(idx8)
                if pending is not None:
                    emit_tail(pending)
                pending = (t, offs, idx8s)
            emit_tail(pending)
    nc.finalize()
    return nc


_PROGRAM_CACHE = {}


def run_on_cores(logits, gumbel, codebook, trace=False, **trace_kwargs):
    if "nc" not in _PROGRAM_CACHE:
        _PROGRAM_CACHE["nc"] = build_program()
    nc = _PROGRAM_CACHE["nc"]
    logits = np.ascontiguousarray(np.asarray(logits, dtype=np.float32)).reshape(
        N_CORES, TOKENS_PER_CORE, N_CODES
    )
    gumbel = np.ascontiguousarray(np.asarray(gumbel, dtype=np.float32)).reshape(
        N_CORES, TOKENS_PER_CORE, N_CODES
    )
    codebook = np.ascontiguousarray(np.asarray(codebook, dtype=np.float32))
    in_maps = [
        {"logits": logits[i], "gumbel": gumbel[i], "codebook": codebook}
        for i in range(N_CORES)
    ]
    res = run_bass_kernel_spmd(
        nc, in_maps, list(range(N_CORES)), trace=trace, **trace_kwargs
    )
    z = np.stack([r["z"] for r in res.results]).astype(np.float32)
    y = np.stack(
        [r["y"].reshape(TOKENS_PER_CORE, N_CODES) for r in res.results]
    ).astype(np.float32)
    return (z, y), res


def kernel(logits, gumbel, codebook):
    (z, y), _ = run_on_cores(logits, gumbel, codebook, trace=False)
    return (z, y)


# revision 16
# speedup vs baseline: 1.4126x; 1.1345x over previous
"""Gumbel-softmax hard VQ codebook kernel for 8 trn2 NeuronCores.

Computes, for inputs logits/gumbel [8, 4096, 2048] and codebook [2048, 256]:
    s   = logits + gumbel
    idx = argmax(s, axis=-1)
    y   = one_hot(idx, 2048) (f32)    # == fwd value of straight-through output
    z   = codebook[idx]               # == einsum(one_hot, codebook)

Sharding: data-parallel over the batch dim (1 batch row of 4096 tokens per core).

Implementation notes:
  - Bulk loads go through HWDGE (nc.sync) with 2 tokens per partition so each
    DMA descriptor moves 16KB contiguous; the add runs on gpsimd (DVE does
    max/max_index).
  - argmax via DVE max/max_index (top-8), first-occurrence tie-break matches
    jnp.argmax.
  - y is written sparsely: run_bass_kernel_spmd pre-zeros (donates zeroed)
    output buffers, so only the 1.0 at (token, idx) is scattered via
    indirect DMA (flat element offsets) - saves 32MB/core of DMA writes.
  - z rows are gathered from the codebook in DRAM via indirect DMA (SWDGE).
"""

import numpy as np

from concourse import bass, bacc, mybir
from concourse.tile import TileContext
from concourse.bass_utils import run_bass_kernel_spmd

P = 128
N_CODES = 2048
CODE_DIM = 256
TOKENS_PER_CORE = 4096
TPP = 2  # tokens per partition per tile
TILE_TOKENS = P * TPP
N_TILES = TOKENS_PER_CORE // TILE_TOKENS
N_CORES = 8

F32 = mybir.dt.float32
I32 = mybir.dt.int32
U32 = mybir.dt.uint32


def build_program() -> bass.Bass:
    nc = bacc.Bacc()
    logits = nc.declare_dram_parameter(
        "logits", [TOKENS_PER_CORE, N_CODES], F32, isOutput=False
    )
    gumbel = nc.declare_dram_parameter(
        "gumbel", [TOKENS_PER_CORE, N_CODES], F32, isOutput=False
    )
    codebook = nc.declare_dram_parameter(
        "codebook", [N_CODES, CODE_DIM], F32, isOutput=False
    )
    # y is flat so the one-hot can be scattered at element granularity.
    y_out = nc.declare_dram_parameter(
        "y", [TOKENS_PER_CORE * N_CODES, 1], F32, isOutput=True
    )
    z_out = nc.declare_dram_parameter(
        "z", [TOKENS_PER_CORE, CODE_DIM], F32, isOutput=True
    )

    with TileContext(nc) as tc:
        with (
            tc.tile_pool(name="const", bufs=1) as cpool,
            tc.tile_pool(name="big", bufs=4) as pool,
            tc.tile_pool(name="small", bufs=6) as spool,
        ):
            ones = cpool.tile([P, 1], F32)
            nc.vector.memset(ones[:], 1.0)

            def emit_tail(item):
                """Indirect scatter/gather + z write for an already-argmaxed
                tile. Emitted one iteration late so the in-order gpsimd queue
                never stalls on the DVE argmax chain of the current tile."""
                t, offs, idx8s = item
                r0 = t * TILE_TOKENS
                rows = slice(r0, r0 + TILE_TOKENS)
                z_tile = pool.tile([P, TPP * CODE_DIM], F32, tag="z")
                for a in range(TPP):
                    nc.gpsimd.indirect_dma_start(
                        out=y_out[:],
                        out_offset=bass.IndirectOffsetOnAxis(
                            ap=offs[a][:, 0:1], axis=0
                        ),
                        in_=ones[:],
                        in_offset=None,
                    )
                    nc.gpsimd.indirect_dma_start(
                        out=z_tile[:, a * CODE_DIM : (a + 1) * CODE_DIM],
                        out_offset=None,
                        in_=codebook[:],
                        in_offset=bass.IndirectOffsetOnAxis(
                            ap=idx8s[a][:, 0:1], axis=0
                        ),
                    )
                nc.sync.dma_start(
                    out=z_out[rows, :].rearrange("(p a) n -> p (a n)", p=P),
                    in_=z_tile[:],
                )

            pending = None
            for t in range(N_TILES):
                r0 = t * TILE_TOKENS
                rows = slice(r0, r0 + TILE_TOKENS)
                # partition p holds tokens r0+2p (cols 0:2048) and r0+2p+1
                sl = pool.tile([P, TPP * N_CODES], F32, tag="sl")
                sg = pool.tile([P, TPP * N_CODES], F32, tag="sg")
                nc.sync.dma_start(
                    out=sl[:],
                    in_=logits[rows, :].rearrange("(p a) n -> p (a n)", p=P),
                )
                nc.sync.dma_start(
                    out=sg[:],
                    in_=gumbel[rows, :].rearrange("(p a) n -> p (a n)", p=P),
                )
                # add in place (s aliases sl), all on DVE: keeping the whole
                # argmax chain on one in-order engine avoids cross-engine
                # ping-pong stalls (gpsimd is left free to issue indirect DMAs)
                s = sl
                nc.vector.tensor_tensor(
                    out=s[:], in0=sl[:], in1=sg[:], op=mybir.AluOpType.add
                )
                offs, idx8s = [], []
                for a in range(TPP):
                    blk = s[:, a * N_CODES : (a + 1) * N_CODES]
                    max8 = spool.tile([P, 8], F32, tag=f"max8_{a}")
                    idx8 = spool.tile([P, 8], U32, tag=f"idx8_{a}")
                    nc.vector.max(max8[:], blk)
                    nc.vector.max_index(idx8[:], max8[:], blk)
                    # flat offset of the argmax element:
                    #   (r0 + 2p + a) * N_CODES + idx
                    rowbase = spool.tile([P, 1], U32, tag=f"rowbase_{a}")
                    nc.gpsimd.iota(
                        rowbase[:],
                        pattern=[[0, 1]],
                        base=(r0 + a) * N_CODES,
                        channel_multiplier=TPP * N_CODES,
                    )
                    off = spool.tile([P, 1], U32, tag=f"off_{a}")
                    nc.vector.tensor_tensor(
                        out=off[:],
                        in0=idx8[:, 0:1],
                        in1=rowbase[:],
                        op=mybir.AluOpType.add,
                    )
                    offs.append(off)
                    idx8s.append(idx8)
                if pending is not None:
                    emit_tail(pending)
                pending = (t, offs, idx8s)
            emit_tail(pending)
    nc.finalize()
    return nc


_PROGRAM_CACHE = {}


def run_on_cores(logits, gumbel, codebook, trace=False, **trace_kwargs):
    if "nc" not in _PROGRAM_CACHE:
        _PROGRAM_CACHE["nc"] = build_program()
    nc = _PROGRAM_CACHE["nc"]
    logits = np.ascontiguousarray(np.asarray(logits, dtype=np.float32)).reshape(
        N_CORES, TOKENS_PER_CORE, N_CODES
    )
    gumbel = np.ascontiguousarray(np.asarray(gumbel, dtype=np.float32)).reshape(
        N_CORES, TOKENS_PER_CORE, N_CODES
    )
    codebook = np.ascontiguousarray(np.asarray(codebook, dtype=np.float32))
    in_maps = [
        {"logits": logits[i], "gumbel": gumbel[i], "codebook": codebook}
        for i in range(N_CORES)
    ]
    res = run_bass_kernel_spmd(
        nc, in_maps, list(range(N_CORES)), trace=trace, **trace_kwargs
    )
    z = np.stack([r["z"] for r in res.results]).astype(np.float32)
    y = np.stack(
        [r["y"].reshape(TOKENS_PER_CORE, N_CODES) for r in res.results]
    ).astype(np.float32)
    return (z, y), res


def kernel(logits, gumbel, codebook):
    (z, y), _ = run_on_cores(logits, gumbel, codebook, trace=False)
    return (z, y)
